# revision 6
# baseline (speedup 1.0000x reference)
"""Trainium2 Bass kernel for a dense transformer block (B=128, T=256, C=384,
H=6, HD=64, FFN=4C), data-parallel over batch across 8 NeuronCores.

Layout strategy (per core, 16 batch elements):
  - LayerNorm runs token-major ([128 tokens, 384] tiles, per-partition stats).
  - Matmul inputs are fp16 (PE runs 1 cycle/row at any free dim, FWL weight
    loads); accumulation is fp32 in PSUM; the residual stream stays fp32.
  - x1/x2/attn are transposed to feature-major with PE identity-matmuls so
    every matmul uses naturally-laid-out operands.
  - Softmax is computed s-major (scoresT = k_f^T q_f per head), exp on the
    scalar engine, causal mask applied by gpsimd affine_select (zero fill).
  - Attention output is computed token-major with the softmax denominator
    folded in as a 65th "ones" column of v; normalization is then a cheap
    per-partition reciprocal + broadcast multiply during PSUM evacuation.
  - LN gains are folded into the adjacent weight matrices host-side; biases
    (all zero in this problem) have exact fallback paths emitted only when
    nonzero at build time.
"""

import sys

sys.path.insert(0, "/opt/trn_rl_repo")

import numpy as np

import concourse.bass as bass
import concourse.tile as tile
from concourse import bacc, mybir
from concourse.bass_utils import run_bass_kernel_spmd

FP32 = mybir.dt.float32
FP16 = mybir.dt.float16
AF = mybir.ActivationFunctionType
ALU = mybir.AluOpType

N_CORES = 8
B, T, C, H, HD = 128, 256, 384, 6, 64
F = 4 * C  # 1536
BC = B // N_CORES  # 16 batches per core
NT = T // 128  # 2 token chunks per batch
NC_ = C // 128  # 3 feature chunks
NF = F // 128  # 12 hidden chunks
EPS = 1e-5
SCALE = HD ** -0.5

_PROGRAM_CACHE = {}


def build_program(flags, repeat=1):
    """flags: (use_qb, use_kb, use_vb, use_bo, use_b2, use_b1) booleans."""
    use_qb, use_kb, use_vb, use_bo, use_b2, use_b1 = flags
    nc = bacc.Bacc("TRN2", target_bir_lowering=False, debug=False,
                   num_devices=N_CORES)

    x_d = nc.dram_tensor("x", [BC, T, C], FP32, kind="ExternalInput").ap()
    wq_d = nc.dram_tensor("wq", [C, C], FP32, kind="ExternalInput").ap()
    wk_d = nc.dram_tensor("wk", [C, C], FP32, kind="ExternalInput").ap()
    wv_d = nc.dram_tensor("wv", [C, C], FP32, kind="ExternalInput").ap()
    wo_d = nc.dram_tensor("wo", [C, C], FP32, kind="ExternalInput").ap()
    w1_d = nc.dram_tensor("w1", [C, F], FP32, kind="ExternalInput").ap()
    w2_d = nc.dram_tensor("w2", [F, C], FP32, kind="ExternalInput").ap()
    id_d = nc.dram_tensor("iden", [128, 128], FP32, kind="ExternalInput").ap()
    qb_d = nc.dram_tensor("qb", [C], FP32, kind="ExternalInput").ap() if use_qb else None
    kb_d = nc.dram_tensor("kb", [C], FP32, kind="ExternalInput").ap() if use_kb else None
    vb_d = nc.dram_tensor("vb", [C], FP32, kind="ExternalInput").ap() if use_vb else None
    bo_d = nc.dram_tensor("bo_r", [C], FP32, kind="ExternalInput").ap() if use_bo else None
    b2_d = nc.dram_tensor("b2_r", [C], FP32, kind="ExternalInput").ap() if use_b2 else None
    b1_d = nc.dram_tensor("b1_r", [F], FP32, kind="ExternalInput").ap() if use_b1 else None
    out_d = nc.dram_tensor("out", [BC, T, C], FP32, kind="ExternalOutput").ap()

    from contextlib import ExitStack
    with tile.TileContext(nc) as tc, ExitStack() as ctx:
        wp = ctx.enter_context(tc.tile_pool(name="wpool", bufs=1))
        ap_ = ctx.enter_context(tc.tile_pool(name="apool", bufs=2))
        sp = ctx.enter_context(tc.tile_pool(name="spool", bufs=4))
        psA = ctx.enter_context(tc.tile_pool(name="psA", bufs=6, space="PSUM"))
        psT = ctx.enter_context(tc.tile_pool(name="psT", bufs=2, space="PSUM"))

        # ---- weights (fp16, cast during SWDGE DMA) ----
        wq_t = wp.tile([128, NC_, C], FP16, tag="wq")
        wk_t = wp.tile([128, NC_, C], FP16, tag="wk")
        wv_t = wp.tile([128, NC_, C], FP16, tag="wv")
        wo_t = wp.tile([128, NC_, C], FP16, tag="wo")
        w1_t = wp.tile([128, NC_, F], FP16, tag="w1")
        w2_t = wp.tile([128, NF, C], FP16, tag="w2")
        iden = wp.tile([128, 128], FP16, tag="iden")
        eps_t = wp.tile([128, 1], FP32, tag="eps")
        nc.vector.memset(eps_t, EPS)
        for wt, wd in ((wq_t, wq_d), (wk_t, wk_d), (wv_t, wv_d), (wo_t, wo_d)):
            nc.gpsimd.dma_start(out=wt, in_=wd.rearrange("(k p) n -> p k n", p=128))
        nc.gpsimd.dma_start(out=w1_t, in_=w1_d.rearrange("(k p) n -> p k n", p=128))
        nc.gpsimd.dma_start(out=w2_t, in_=w2_d.rearrange("(m p) n -> p m n", p=128))
        nc.gpsimd.dma_start(out=iden, in_=id_d)

        ones_row = None
        if use_vb or use_bo or use_b2:
            ones_row = wp.tile([1, 128], FP16, tag="ones_row")
            nc.vector.memset(ones_row, 1.0)
        qb_t = kb_t = None
        if use_qb:
            qb_t = wp.tile([128, NC_], FP32, tag="qb")
            nc.sync.dma_start(out=qb_t, in_=qb_d.rearrange("(m p) -> p m", p=128))
        if use_kb:
            kb_t = wp.tile([128, NC_], FP32, tag="kb")
            nc.sync.dma_start(out=kb_t, in_=kb_d.rearrange("(m p) -> p m", p=128))
        vb_t = bo_t = b2_t = b1_t = None
        if use_vb:
            vb_t = wp.tile([1, C], FP16, tag="vb")
            nc.gpsimd.dma_start(out=vb_t, in_=vb_d[None, :])
        if use_bo:
            bo_t = wp.tile([1, C], FP16, tag="bo")
            nc.gpsimd.dma_start(out=bo_t, in_=bo_d[None, :])
        if use_b2:
            b2_t = wp.tile([1, C], FP16, tag="b2")
            nc.gpsimd.dma_start(out=b2_t, in_=b2_d[None, :])
        if use_b1:
            b1_t = wp.tile([128, NF], FP32, tag="b1")
            nc.sync.dma_start(out=b1_t, in_=b1_d.rearrange("(m p) -> p m", p=128))

        for b in [bb for _ in range(repeat) for bb in range(BC)]:
            # ---- load x [128, 2, 384] fp32 ----
            x_t = ap_.tile([128, NT, C], FP32, tag="x")
            nc.sync.dma_start(out=x_t, in_=x_d[b].rearrange("(i p) c -> p i c", p=128))

            # ---- LN1 -> x1 fp16 token-major ----
            mv = sp.tile([128, NT, 2], FP32, tag="mv")
            for i in range(NT):
                st = sp.tile([128, 6], FP32, tag="st")
                nc.vector.bn_stats(out=st, in_=x_t[:, i, :])
                nc.vector.bn_aggr(out=mv[:, i, :], in_=st)
            lnv = sp.tile([128, NT, 1], FP32, tag="lnv")
            nc.scalar.activation(out=lnv, in_=mv[:, :, 1:2], func=AF.Ln, bias=eps_t)
            rstd = sp.tile([128, NT, 1], FP32, tag="rstd")
            nc.scalar.activation(out=rstd, in_=lnv, func=AF.Exp, scale=-0.5)
            x1 = ap_.tile([128, NT, C], FP16, tag="x1")
            for i in range(NT):
                nc.vector.tensor_scalar(
                    out=x1[:, i, :], in0=x_t[:, i, :],
                    scalar1=mv[:, i, 0:1], scalar2=rstd[:, i, :],
                    op0=ALU.subtract, op1=ALU.mult)

            # ---- x1 -> feature-major x1f [128c, 3, 256t] ----
            p_tr = psT.tile([128, NC_, T], FP16, tag="tr")
            for i in range(NT):
                for j in range(NC_):
                    nc.tensor.transpose(p_tr[:, j, 128 * i:128 * (i + 1)],
                                        x1[:, i, 128 * j:128 * (j + 1)], iden)
            x1f = ap_.tile([128, NC_, T], FP16, tag="x1f")
            nc.vector.tensor_copy(x1f, p_tr)

            # ---- q, k projections (feature-major out) ----
            q_f = ap_.tile([128, NC_, T], FP16, tag="q_f")
            k_f = ap_.tile([128, NC_, T], FP16, tag="k_f")
            for dst, wt, bias_t in ((q_f, wq_t, qb_t), (k_f, wk_t, kb_t)):
                for m in range(NC_):
                    pq = psA.tile([128, T], FP32, tag="a")
                    for kk in range(NC_):
                        nc.tensor.matmul(pq, wt[:, kk, 128 * m:128 * (m + 1)],
                                         x1f[:, kk, :],
                                         start=(kk == 0), stop=(kk == NC_ - 1))
                    if bias_t is not None:
                        nc.scalar.activation(out=dst[:, m, :], in_=pq, func=AF.Identity,
                                             bias=bias_t[:, m:m + 1])
                    else:
                        nc.scalar.copy(out=dst[:, m, :], in_=pq)

            # ---- v projection (token-major, interleaved with ones col) ----
            v_t = ap_.tile([128, NT, H, HD + 1], FP16, tag="v_t")
            for i in range(NT):
                pv = psA.tile([128, C], FP32, tag="a")
                for kk in range(NC_):
                    nc.tensor.matmul(pv, x1f[:, kk, 128 * i:128 * (i + 1)],
                                     wv_t[:, kk, :], start=(kk == 0),
                                     stop=(kk == NC_ - 1 and vb_t is None))
                if vb_t is not None:
                    nc.tensor.matmul(pv, ones_row, vb_t, start=False, stop=True,
                                     skip_group_check=True)
                nc.vector.tensor_copy(v_t[:, i, :, 0:HD],
                                      pv.rearrange("p (h d) -> p h d", h=H))
                nc.vector.memset(v_t[:, i, :, HD:HD + 1], 1.0)

            # ---- scores (s-major), exp, causal mask ----
            w_t = ap_.tile([128, H, NT, T], FP16, tag="w_t")
            for h in range(H):
                j, r = h // 2, (h % 2) * 64
                ps = psA.tile([128, NT, T], FP32, tag="a")
                nc.tensor.matmul(ps[:, 0, :], k_f[r:r + 64, j, 0:128],
                                 q_f[r:r + 64, j, :], start=True, stop=True)
                nc.tensor.matmul(ps[:, 1, 128:T], k_f[r:r + 64, j, 128:T],
                                 q_f[r:r + 64, j, 128:T], start=True, stop=True)
                nc.scalar.activation(out=w_t[:, h, :, :], in_=ps, func=AF.Exp,
                                     scale=SCALE)
                for si in range(NT):
                    nc.gpsimd.affine_select(
                        out=w_t[:, h, si, :], in_=w_t[:, h, si, :],
                        pattern=[[1, T]], compare_op=ALU.is_ge, fill=0.0,
                        base=-128 * si, channel_multiplier=-1)

            # ---- attention (token-major, Z in column 64) ----
            a_tok = ap_.tile([128, NT, C], FP16, tag="a_tok")
            for i in range(NT):
                pa = psA.tile([128, H, HD + 1], FP32, tag="a")
                for h in range(H):
                    for si in range(i + 1):
                        nc.tensor.matmul(pa[:, h, :],
                                         w_t[:, h, si, 128 * i:128 * (i + 1)],
                                         v_t[:, si, h, :],
                                         start=(si == 0), stop=(si == i))
                r_t = sp.tile([128, H, 1], FP32, tag="r_t")
                nc.vector.reciprocal(r_t, pa[:, :, HD:HD + 1])
                bcast = bass.AP(tensor=r_t.tensor, offset=r_t.offset,
                                ap=[r_t.ap[0], [1, H], [0, HD]])
                nc.vector.tensor_tensor(out=a_tok[:, i, :].rearrange(
                    "p (h d) -> p h d", h=H), in0=pa[:, :, 0:HD], in1=bcast,
                    op=ALU.mult)

            # ---- attn -> feature-major ----
            p_at = psT.tile([128, NC_, T], FP16, tag="tr")
            for i in range(NT):
                for j in range(NC_):
                    nc.tensor.transpose(p_at[:, j, 128 * i:128 * (i + 1)],
                                        a_tok[:, i, 128 * j:128 * (j + 1)], iden)
            a_f = ap_.tile([128, NC_, T], FP16, tag="a_f")
            nc.vector.tensor_copy(a_f, p_at)

            # ---- output projection + residual -> y (fp32) ----
            y_t = ap_.tile([128, NT, C], FP32, tag="y_t")
            for i in range(NT):
                py = psA.tile([128, C], FP32, tag="a")
                for kk in range(NC_):
                    nc.tensor.matmul(py, a_f[:, kk, 128 * i:128 * (i + 1)],
                                     wo_t[:, kk, :], start=(kk == 0),
                                     stop=(kk == NC_ - 1 and bo_t is None))
                if bo_t is not None:
                    nc.tensor.matmul(py, ones_row, bo_t, start=False, stop=True,
                                     skip_group_check=True)
                nc.vector.tensor_tensor(out=y_t[:, i, :], in0=py, in1=x_t[:, i, :],
                                        op=ALU.add)

            # ---- LN2 -> x2 fp16 ----
            mv2 = sp.tile([128, NT, 2], FP32, tag="mv")
            for i in range(NT):
                st2 = sp.tile([128, 6], FP32, tag="st")
                nc.vector.bn_stats(out=st2, in_=y_t[:, i, :])
                nc.vector.bn_aggr(out=mv2[:, i, :], in_=st2)
            lnv2 = sp.tile([128, NT, 1], FP32, tag="lnv")
            nc.scalar.activation(out=lnv2, in_=mv2[:, :, 1:2], func=AF.Ln, bias=eps_t)
            rstd2 = sp.tile([128, NT, 1], FP32, tag="rstd")
            nc.scalar.activation(out=rstd2, in_=lnv2, func=AF.Exp, scale=-0.5)
            x2 = ap_.tile([128, NT, C], FP16, tag="x2")
            for i in range(NT):
                nc.vector.tensor_scalar(
                    out=x2[:, i, :], in0=y_t[:, i, :],
                    scalar1=mv2[:, i, 0:1], scalar2=rstd2[:, i, :],
                    op0=ALU.subtract, op1=ALU.mult)

            # ---- x2 -> feature-major ----
            p_x2 = psT.tile([128, NC_, T], FP16, tag="tr")
            for i in range(NT):
                for j in range(NC_):
                    nc.tensor.transpose(p_x2[:, j, 128 * i:128 * (i + 1)],
                                        x2[:, i, 128 * j:128 * (j + 1)], iden)
            x2f = ap_.tile([128, NC_, T], FP16, tag="x2f")
            nc.vector.tensor_copy(x2f, p_x2)

            # ---- FFN1 + ReLU -> h_t fp16 (feature-major) ----
            h_t = ap_.tile([128, NF, T], FP16, tag="h_t")
            for mp in range(NF // 2):
                ph = psA.tile([128, 2, T], FP32, tag="a")
                for sub in range(2):
                    m = 2 * mp + sub
                    for kk in range(NC_):
                        nc.tensor.matmul(ph[:, sub, :],
                                         w1_t[:, kk, 128 * m:128 * (m + 1)],
                                         x2f[:, kk, :],
                                         start=(kk == 0), stop=(kk == NC_ - 1))
                if b1_t is not None:
                    for sub in range(2):
                        m = 2 * mp + sub
                        nc.vector.tensor_scalar(
                            out=h_t[:, m, :], in0=ph[:, sub, :],
                            scalar1=b1_t[:, m:m + 1], scalar2=0.0,
                            op0=ALU.add, op1=ALU.max)
                elif mp % 2 == 0:
                    nc.vector.tensor_scalar(
                        out=h_t[:, 2 * mp:2 * mp + 2, :], in0=ph,
                        scalar1=0.0, scalar2=None, op0=ALU.max)
                else:
                    nc.scalar.activation(out=h_t[:, 2 * mp:2 * mp + 2, :], in_=ph,
                                         func=AF.Relu)

            # ---- FFN2 + residual -> out ----
            o_t = ap_.tile([128, NT, C], FP32, tag="o_t")
            for i in range(NT):
                po = psA.tile([128, C], FP32, tag="a")
                for m in range(NF):
                    nc.tensor.matmul(po, h_t[:, m, 128 * i:128 * (i + 1)],
                                     w2_t[:, m, :], start=(m == 0),
                                     stop=(m == NF - 1 and b2_t is None))
                if b2_t is not None:
                    nc.tensor.matmul(po, ones_row, b2_t, start=False, stop=True,
                                     skip_group_check=True)
                nc.vector.tensor_tensor(out=o_t[:, i, :], in0=po, in1=y_t[:, i, :],
                                        op=ALU.add)

            nc.sync.dma_start(out=out_d[b].rearrange("(i p) c -> p i c", p=128),
                              in_=o_t)

    nc.compile()
    return nc


def _prep(inputs):
    """Host-side preprocessing: fold LN gains into weights, compute effective
    biases, return (flags, extra per-core-constant input map)."""
    g1 = np.asarray(inputs["ln1_g"], np.float32)
    b1ln = np.asarray(inputs["ln1_b"], np.float32)
    g2 = np.asarray(inputs["ln2_g"], np.float32)
    b2ln = np.asarray(inputs["ln2_b"], np.float32)
    Wq = np.asarray(inputs["Wq"], np.float32).transpose(1, 0, 2).reshape(C, C)
    Wk = np.asarray(inputs["Wk"], np.float32).transpose(1, 0, 2).reshape(C, C)
    Wv = np.asarray(inputs["Wv"], np.float32).transpose(1, 0, 2).reshape(C, C)
    Wo = np.asarray(inputs["Wo"], np.float32)
    bo = np.asarray(inputs["bo"], np.float32)
    W1 = np.asarray(inputs["W1"], np.float32)
    b1 = np.asarray(inputs["b1"], np.float32)
    W2 = np.asarray(inputs["W2"], np.float32)
    b2 = np.asarray(inputs["b2"], np.float32)

    qb = b1ln @ Wq
    kb = b1ln @ Wk
    vb = b1ln @ Wv
    b1_eff = b1 + b2ln @ W1

    const = {
        "wq": g1[:, None] * Wq, "wk": g1[:, None] * Wk, "wv": g1[:, None] * Wv,
        "wo": Wo, "w1": g2[:, None] * W1, "w2": W2,
        "iden": np.eye(128, dtype=np.float32),
    }
    flags = (bool(np.any(qb)), bool(np.any(kb)), bool(np.any(vb)),
             bool(np.any(bo)), bool(np.any(b2)), bool(np.any(b1_eff)))
    if flags[0]: const["qb"] = qb
    if flags[1]: const["kb"] = kb
    if flags[2]: const["vb"] = vb
    if flags[3]: const["bo_r"] = bo
    if flags[4]: const["b2_r"] = b2
    if flags[5]: const["b1_r"] = b1_eff
    return flags, const


def kernel(**inputs):
    flags, const = _prep(inputs)
    if flags not in _PROGRAM_CACHE:
        _PROGRAM_CACHE[flags] = build_program(flags)
    nc = _PROGRAM_CACHE[flags]

    x = np.ascontiguousarray(np.asarray(inputs["x"], np.float32))
    in_maps = [dict(const, x=x[c * BC:(c + 1) * BC]) for c in range(N_CORES)]
    res = run_bass_kernel_spmd(nc, in_maps, core_ids=list(range(N_CORES)))
    return np.concatenate([res.results[c]["out"] for c in range(N_CORES)], axis=0)


if __name__ == "__main__":
    rng = np.random.default_rng(0)
    demo = {
        "x": rng.standard_normal((B, T, C), dtype=np.float32),
        "ln1_g": np.ones(C, np.float32), "ln1_b": np.zeros(C, np.float32),
        "Wq": rng.standard_normal((H, C, HD), dtype=np.float32) / np.sqrt(C),
        "Wk": rng.standard_normal((H, C, HD), dtype=np.float32) / np.sqrt(C),
        "Wv": rng.standard_normal((H, C, HD), dtype=np.float32) / np.sqrt(C),
        "Wo": rng.standard_normal((C, C), dtype=np.float32) / np.sqrt(C),
        "bo": np.zeros(C, np.float32),
        "ln2_g": np.ones(C, np.float32), "ln2_b": np.zeros(C, np.float32),
        "W1": rng.standard_normal((C, F), dtype=np.float32) / np.sqrt(C),
        "b1": np.zeros(F, np.float32),
        "W2": rng.standard_normal((F, C), dtype=np.float32) / np.sqrt(F),
        "b2": np.zeros(C, np.float32),
    }
    out = kernel(**demo)
    print("out", out.shape, out.dtype, float(np.abs(out).max()))
